# revision 28
# baseline (speedup 1.0000x reference)
"""HalfKP input layer (embedding_lookup) on 8 Trainium2 NeuronCores.

Reference computation (B=1024, K=64, F=640, C=256):
    p = piece_positions.reshape(B, 640).astype(f32)          # values in {0,1}
    Wg = input_weights[king_positions]                       # (B, 2, 641, 256)
    out[b] = sum_f p[b,f] * (Wg[b,0,f,:] + Wg[b,1,f,:])
             + Wg[b,0,640,:] + Wg[b,1,640,:] + bias

Strategy — SINGLE launch, channel-sharded (launch fixed cost on this stack
is ~13-16us, so the two-launch host-routed design pays it twice):
  * Core c owns output channels [32c, 32c+32).  It computes ALL 2048
    (sample, king-slot) pair rows restricted to its 32 channels, so the
    pair combine (rowA + rowB) is core-local — no cross-core traffic.
  * Table read stays minimal: each core reads only its 32-channel bf16
    slice (2.62 MB) -> the table is read once in aggregate.
  * HWDGE descriptor generation paces DMAs at ~40ns/descriptor (one per
    partition-run), so weights/features stream in 3 pieces of ~7KB runs
    on the two HWDGE queues (~350 GB/s aggregate); small tensors ride
    the independent SWDGE (gpsimd) queue.
  * Main compute is transposed: psum[32ch, col] += W[k,ch].T @ feats.
    One psum tile [128, 512]: band b = cols [512b, 512b+512) at
    partitions [32b, 32b+32); col-tiled matmuls from different bands run
    concurrently.  A K=2 matmul per subgroup adds the row-640 extra
    (all pairs) + bias (slot-A pairs) from a tiny [2, 64*32] table.
  * Pair combine ON THE PE (GPSIMD ap_gather measured ~27ns/index =
    55us for 2048 — unusable): one-hot selection tiles
    S_c[128pair, 1024sample] = (sampleof == iota) are built by the DVE
    early (hidden under the DMA window); per 128-pair chunk the bf16 row
    block is PE-transposed, then 32 pairing matmuls
    psumOut += rows_c.T @ S_c accumulate rowA+rowB per sample directly,
    4 chunks concurrently via col-tiling; a DVE chain sums the 4 bands.
  * Host transposes per-core outT[32, 1024] slices into (1024, 256) —
    pure indexing.
"""

import os
from contextlib import ExitStack

import numpy as np
import ml_dtypes

import concourse.bass as bass
import concourse.tile as tile
from concourse import bacc, mybir
from concourse.bass_utils import run_bass_kernel_spmd
from concourse.masks import make_identity

B = 1024
K = 64
F = 640
C = 256
NCORES = 8
CH = C // NCORES       # 32 channels per core
FCH = F // 128         # 5 feature chunks of 128
P = 128
NCOL = 2 * B           # 2048 pair columns
BINW = 512             # psum band width (one fp32 bank)
NBAND = NCOL // BINW   # 4
NCHUNK = NCOL // P     # 16 pair chunks for the pairing matmuls
NPIECE = 3             # DMA pieces for weights/features
FEDGE = [0, 512, 1408, 2048]
KEDGE = [0, 16, 42, 64]

BF16 = ml_dtypes.bfloat16

# Exposed for test harnesses
LAST_RESULTS = []
LAST_EXEC_NS = None

_cache = {}


def _prep(king_positions):
    """Group the 2048 (sample, slot) pairs by king into a 2048-long column
    order; split groups at psum-band (512) and DMA-piece edges."""
    kings = np.asarray(king_positions).astype(np.int64)  # (B, 2)
    groups = [[] for _ in range(K)]
    for b in range(B):
        groups[kings[b, 0]].append((b, 0))
        groups[kings[b, 1]].append((b, 1))

    edges = sorted(set([b * BINW for b in range(NBAND + 1)] + FEDGE))

    order = []       # col -> (b, s)
    subgroups = []   # (king, c0, n)
    col = 0
    for k in range(K):
        g = groups[k]
        i = 0
        while i < len(g):
            nxt = min(e for e in edges if e > col)
            n = min(len(g) - i, nxt - col)
            subgroups.append((k, col, n))
            order.extend(g[i : i + n])
            i += n
            col += n
    assert col == NCOL

    pos = np.empty((B, 2), dtype=np.int64)
    for c, (b, s) in enumerate(order):
        pos[b, s] = c
    return order, tuple(subgroups), pos


def _build(subgroups):
    nc = bacc.Bacc(
        "TRN2", target_bir_lowering=False, debug=False, num_devices=NCORES
    )
    dt = mybir.dt

    w_ins = [
        nc.dram_tensor(
            f"w_in{i}", [P, KEDGE[i + 1] - KEDGE[i], FCH, CH], dt.bfloat16,
            kind="ExternalInput",
        )
        for i in range(NPIECE)
    ]
    f_ins = [
        nc.dram_tensor(
            f"f_in{i}", [P, FCH, FEDGE[i + 1] - FEDGE[i]], dt.bfloat16,
            kind="ExternalInput",
        )
        for i in range(NPIECE)
    ]
    koh_d = nc.dram_tensor("koh", [K + 1, NCOL], dt.bfloat16, kind="ExternalInput")
    eb65_d = nc.dram_tensor("eb65", [K + 1, CH], dt.bfloat16, kind="ExternalInput")
    sofT_d = nc.dram_tensor("sofT", [NCHUNK, P], dt.float16, kind="ExternalInput")
    outT = nc.dram_tensor("outT", [CH, B], dt.float32, kind="ExternalOutput")

    with tile.TileContext(nc) as tc, ExitStack() as ctx:
        const_pool = ctx.enter_context(tc.tile_pool(name="const", bufs=1))
        psum_pool = ctx.enter_context(tc.tile_pool(name="psum", bufs=1, space="PSUM"))

        # latency-critical smalls + first weight piece on the sync HWDGE
        # queue (otherwise idle); the iota + later weight pieces ride the
        # SWDGE queue (slower drain, needed later)
        sof16_sb = const_pool.tile([P, NCHUNK], dt.float16)
        nc.sync.dma_start(out=sof16_sb[:], in_=sofT_d.ap(), transpose=True)
        srow_sb = const_pool.tile([P, B], dt.float32)
        nc.gpsimd.iota(
            srow_sb[:], pattern=[[1, B]], base=0, channel_multiplier=0,
            allow_small_or_imprecise_dtypes=True,
        )
        w_sb = []
        for i in range(NPIECE):
            nk = KEDGE[i + 1] - KEDGE[i]
            wt = const_pool.tile([P, nk * FCH * CH], dt.bfloat16, name=f"wt{i}")
            (nc.sync if i == 0 else nc.gpsimd).dma_start(
                out=wt[:], in_=w_ins[i].ap().rearrange("p k f c -> p (k f c)")
            )
            w_sb.append(wt)
        koh_sb = const_pool.tile([K + 1, NCOL], dt.bfloat16)
        nc.sync.dma_start(out=koh_sb[:], in_=koh_d.ap())
        eb65_sb = const_pool.tile([K + 1, CH], dt.bfloat16)
        nc.sync.dma_start(out=eb65_sb[:], in_=eb65_d.ap())
        ident_sb = const_pool.tile([CH, CH], dt.bfloat16)
        make_identity(nc, ident_sb[:])
        nsof_sb = const_pool.tile([P, NCHUNK], dt.float32)
        srow16_sb = const_pool.tile([P, B], dt.float16)
        with tc.high_priority():
            nc.vector.tensor_scalar(
                out=nsof_sb[:], in0=sof16_sb[:], scalar1=-1.0, scalar2=None,
                op0=mybir.AluOpType.mult,
            )
            nc.vector.tensor_copy(srow16_sb[:], srow_sb[:])

        # one-hot pairing tiles S_c[p, b] = (sampleof(128c+p) == b), bf16
        s_sb = const_pool.tile([P, NCHUNK * B], dt.bfloat16)
        sact_sb = const_pool.tile([P, B], dt.float32)

        def emit_sgen_dve(cs):
            for c in cs:
              with tc.high_priority():
                nc.vector.tensor_tensor(
                    out=s_sb[:, c * B : (c + 1) * B],
                    in0=sof16_sb[:, c : c + 1].to_broadcast([P, B]),
                    in1=srow16_sb[:],
                    op=mybir.AluOpType.is_equal,
                )

        def emit_sgen_act(cs):
            # S_c = relu(1 - |srow - sof_c|): exact one-hot for integers
            for c in cs:
                nc.scalar.activation(
                    out=sact_sb[:],
                    in_=srow_sb[:],
                    func=mybir.ActivationFunctionType.Abs,
                    bias=nsof_sb[:, c : c + 1],
                    scale=1.0,
                )
                nc.scalar.activation(
                    out=s_sb[:, c * B : (c + 1) * B],
                    in_=sact_sb[:],
                    func=mybir.ActivationFunctionType.Relu,
                    bias=1.0,
                    scale=-1.0,
                )

        # feature stream on the two HWDGE queues (triggers must precede the
        # ACT S-gen ops in the scalar sequencer's FIFO)
        f_sb = []
        for i in range(NPIECE):
            nf = FEDGE[i + 1] - FEDGE[i]
            ft = const_pool.tile([P, FCH * nf], dt.bfloat16, name=f"ft{i}")
            nc.scalar.dma_start(
                out=ft[:], in_=f_ins[i].ap().rearrange("p f c -> p (f c)")
            )
            f_sb.append(ft)



        # psum: main accumulators (one bank per band; band b uses
        # partitions [32b, 32b+32) so its matmuls col-tile concurrently),
        # transpose staging, pairing output
        accs = [
            psum_pool.tile([P, BINW], dt.float32, space="PSUM", name=f"acc{i}")
            for i in range(NBAND)
        ]
        tpsum = psum_pool.tile([P, NCHUNK * CH], dt.bfloat16, space="PSUM")
        pout = psum_pool.tile([P, B], dt.float32, space="PSUM")

        rows_sb = const_pool.tile([CH, NCOL], dt.bfloat16)
        rcT_sb = const_pool.tile([P, NCHUNK * CH], dt.bfloat16)

        def piece_of(c0):
            for i in range(NPIECE):
                if FEDGE[i] <= c0 < FEDGE[i + 1]:
                    return i
            raise AssertionError

        by_piece_band = {}
        for k, c0, n in subgroups:
            by_piece_band.setdefault(
                (piece_of(c0), c0 // BINW), []
            ).append((k, c0, n))

        band_started = [False] * NBAND

        def emit_mains(pc):
            bands = sorted(b for (p_, b) in by_piece_band if p_ == pc)
            lists = [list(by_piece_band[(pc, b)]) for b in bands]
            li = 0
            while any(lists):
                if lists[li % len(lists)]:
                    k, c0, n = lists[li % len(lists)].pop(0)
                    band = c0 // BINW
                    off = c0 % BINW
                    poff = c0 - FEDGE[pc]
                    npc = FEDGE[pc + 1] - FEDGE[pc]
                    wc = next(
                        i for i in range(NPIECE) if KEDGE[i] <= k < KEDGE[i + 1]
                    )
                    kk = k - KEDGE[wc]
                    for ch in range(FCH):
                        st = not band_started[band]
                        band_started[band] = True
                        nc.tensor.matmul(
                            out=accs[band][32 * band : 32 * band + CH, off : off + n],
                            lhsT=w_sb[wc][
                                :, (kk * FCH + ch) * CH : (kk * FCH + ch + 1) * CH
                            ],
                            rhs=f_sb[pc][:, ch * npc + poff : ch * npc + poff + n],
                            start=st,
                            stop=False,
                            tile_position=(0, 32 * band),
                        )
                li += 1

        def emit_fold(band):
            # psum band -> flat bf16 rows (partition-shifted psum read)
            nc.vector.tensor_copy(
                rows_sb[:, band * BINW : (band + 1) * BINW],
                accs[band][32 * band : 32 * band + CH, :],
            )

        def emit_transposes(cs):
            for c in cs:
                nc.tensor.transpose(
                    out=tpsum[:, c * CH : (c + 1) * CH],
                    in_=rows_sb[:, c * P : (c + 1) * P],
                    identity=ident_sb[:],
                )

        def emit_tcopy(g):
            nc.vector.tensor_copy(
                rcT_sb[:, g * 4 * CH : (g + 1) * 4 * CH],
                tpsum[:, g * 4 * CH : (g + 1) * 4 * CH],
            )

        def emit_pairing(cs):
            for c in cs:
                band = c % NBAND
                for h in range(2):
                    nc.tensor.matmul(
                        out=pout[32 * band : 32 * band + CH, h * BINW : (h + 1) * BINW],
                        lhsT=rcT_sb[:, c * CH : (c + 1) * CH],
                        rhs=s_sb[:, c * B + h * BINW : c * B + (h + 1) * BINW],
                        start=(c // NBAND == 0),
                        stop=(c // NBAND == NBAND - 1),
                        tile_position=(0, 32 * band),
                        skip_group_check=True,
                    )

        def emit_extras(bands):
            # row-640 extra (all pairs) + bias (slot-A pairs): one K=65
            # matmul per band closes the band's accumulation
            for band in bands:
                nc.tensor.matmul(
                    out=accs[band][32 * band : 32 * band + CH, :],
                    lhsT=eb65_sb[:, :],
                    rhs=koh_sb[:, band * BINW : (band + 1) * BINW],
                    start=False,
                    stop=True,
                    tile_position=(0, 32 * band),
                )

        # pipeline: mains by piece; band work as its columns complete
        emit_sgen_act(range(11, NCHUNK))   # ACT, parallel to everything
        emit_sgen_dve(range(0, 6))
        emit_mains(0)
        emit_extras([0])
        emit_fold(0)
        emit_sgen_dve(range(6, 8))
        emit_mains(1)
        emit_extras([1])
        emit_fold(1)
        emit_transposes(range(0, 8))
        emit_tcopy(0)
        emit_sgen_dve(range(8, 11))
        emit_tcopy(1)
        emit_pairing(range(0, 8))
        emit_mains(2)
        emit_extras([2, 3])
        emit_fold(2)
        emit_fold(3)
        emit_transposes(range(8, 16))
        emit_tcopy(2)
        emit_tcopy(3)
        emit_pairing(range(8, 16))

        # sum the 4 pairing bands (partition-shifted psum reads)
        t0_sb = const_pool.tile([CH, B], dt.bfloat16)
        nc.scalar.copy(t0_sb[:], pout[0:CH, :])
        t1_sb = const_pool.tile([CH, B], dt.bfloat16)
        nc.vector.tensor_tensor(
            out=t1_sb[:], in0=pout[CH : 2 * CH, :], in1=t0_sb[:],
            op=mybir.AluOpType.add,
        )
        t2_sb = const_pool.tile([CH, B], dt.bfloat16)
        nc.vector.tensor_tensor(
            out=t2_sb[:], in0=pout[2 * CH : 3 * CH, :], in1=t1_sb[:],
            op=mybir.AluOpType.add,
        )
        outT_sb = const_pool.tile([CH, B], dt.float32)
        nc.vector.tensor_tensor(
            out=outT_sb[:], in0=pout[3 * CH : 4 * CH, :], in1=t2_sb[:],
            op=mybir.AluOpType.add,
        )
        nc.sync.dma_start(out=outT.ap(), in_=outT_sb[:])

    nc.compile()
    return nc


def kernel(piece_positions, king_positions, input_weights, bias):
    global LAST_RESULTS, LAST_EXEC_NS

    p_flat = np.asarray(piece_positions).reshape(B, F)
    w_full = np.ascontiguousarray(np.asarray(input_weights), dtype=np.float32)
    bias_np = np.asarray(bias, dtype=np.float32)

    order, subgroups, pos = _prep(king_positions)

    if subgroups not in _cache:
        _cache[subgroups] = _build(subgroups)
    nc = _cache[subgroups]

    w_bf = w_full.astype(BF16)  # (K, 641, C)

    # features in pair-column order: (2048, 640) -> (128, 5, 2048) bf16
    bs = np.array([b for b, _ in order], dtype=np.int64)
    featsT = (
        p_flat[bs].astype(np.float32).reshape(NCOL, FCH, 128).transpose(2, 1, 0)
    ).astype(BF16)
    f_pieces = [
        np.ascontiguousarray(featsT[:, :, FEDGE[i] : FEDGE[i + 1]])
        for i in range(NPIECE)
    ]

    # koh: one-hot king per column (row-640 extra) + slot-A row (bias once)
    koh = np.zeros((K + 1, NCOL), dtype=np.float32)
    for c0, (b, s) in enumerate(order):
        koh[np.asarray(king_positions)[b, s], c0] = 1.0
        if s == 0:
            koh[K, c0] = 1.0
    koh = koh.astype(BF16)

    # pairing metadata: sample index of each pair column, and iota row
    sofT = np.empty((NCHUNK, P), dtype=np.float16)
    for c0, (b, s) in enumerate(order):
        sofT[c0 // P, c0 % P] = float(b)

    in_maps = []
    for c in range(NCORES):
        chs = slice(c * CH, (c + 1) * CH)
        w_c = (
            w_bf[:, :F, chs].reshape(K, FCH, 128, CH).transpose(2, 0, 1, 3)
        )  # (128, K, FCH, CH)
        eb65 = np.zeros((K + 1, CH), dtype=np.float32)
        eb65[:K] = w_full[:, F, chs]
        eb65[K] = bias_np[chs]
        m = {
            "koh": koh,
            "eb65": np.ascontiguousarray(eb65).astype(BF16),
            "sofT": sofT,
        }
        for i in range(NPIECE):
            m[f"w_in{i}"] = np.ascontiguousarray(w_c[:, KEDGE[i] : KEDGE[i + 1]])
            m[f"f_in{i}"] = f_pieces[i]
        in_maps.append(m)

    do_trace = bool(int(os.environ.get("KERNEL_TRACE", "0")))
    trace_kw = dict(
        trace=do_trace, trace_cores=list(range(NCORES)) if do_trace else None
    )

    res = run_bass_kernel_spmd(nc, in_maps, list(range(NCORES)), **trace_kw)

    LAST_RESULTS = [res]
    LAST_EXEC_NS = res.exec_time_ns

    out = np.empty((B, C), dtype=np.float32)
    for c in range(NCORES):
        out[:, c * CH : (c + 1) * CH] = res.results[c]["outT"].T
    return out


# revision 29
# speedup vs baseline: 1.1011x; 1.1011x over previous
"""HalfKP input layer (embedding_lookup) on 8 Trainium2 NeuronCores.

Reference computation (B=1024, K=64, F=640, C=256):
    p = piece_positions.reshape(B, 640).astype(f32)          # values in {0,1}
    Wg = input_weights[king_positions]                       # (B, 2, 641, 256)
    out[b] = sum_f p[b,f] * (Wg[b,0,f,:] + Wg[b,1,f,:])
             + Wg[b,0,640,:] + Wg[b,1,640,:] + bias

Strategy — SINGLE launch, channel-sharded (launch fixed cost on this stack
is ~13-16us, so the two-launch host-routed design pays it twice):
  * Core c owns output channels [32c, 32c+32).  It computes ALL 2048
    (sample, king-slot) pair rows restricted to its 32 channels, so the
    pair combine (rowA + rowB) is core-local — no cross-core traffic.
  * Table read stays minimal: each core reads only its 32-channel bf16
    slice (2.62 MB) -> the table is read once in aggregate.
  * HWDGE descriptor generation paces DMAs at ~40ns/descriptor (one per
    partition-run), so weights/features stream in 3 pieces of ~7KB runs
    on the two HWDGE queues (~350 GB/s aggregate); small tensors ride
    the independent SWDGE (gpsimd) queue.
  * Main compute is transposed: psum[32ch, col] += W[k,ch].T @ feats.
    One psum tile [128, 512]: band b = cols [512b, 512b+512) at
    partitions [32b, 32b+32); col-tiled matmuls from different bands run
    concurrently.  A K=2 matmul per subgroup adds the row-640 extra
    (all pairs) + bias (slot-A pairs) from a tiny [2, 64*32] table.
  * Pair combine ON THE PE (GPSIMD ap_gather measured ~27ns/index =
    55us for 2048 — unusable): one-hot selection tiles
    S_c[128pair, 1024sample] = (sampleof == iota) are built by the DVE
    early (hidden under the DMA window); per 128-pair chunk the bf16 row
    block is PE-transposed, then 32 pairing matmuls
    psumOut += rows_c.T @ S_c accumulate rowA+rowB per sample directly,
    4 chunks concurrently via col-tiling; a DVE chain sums the 4 bands.
  * Host transposes per-core outT[32, 1024] slices into (1024, 256) —
    pure indexing.
"""

import os
from contextlib import ExitStack

import numpy as np
import ml_dtypes

import concourse.bass as bass
import concourse.tile as tile
from concourse import bacc, mybir
from concourse.bass_utils import run_bass_kernel_spmd
from concourse.masks import make_identity

B = 1024
K = 64
F = 640
C = 256
NCORES = 8
CH = C // NCORES       # 32 channels per core
FCH = F // 128         # 5 feature chunks of 128
P = 128
NCOL = 2 * B           # 2048 pair columns
BINW = 512             # psum band width (one fp32 bank)
NBAND = NCOL // BINW   # 4
NCHUNK = NCOL // P     # 16 pair chunks for the pairing matmuls
NPIECE = 3             # DMA pieces for weights/features
FEDGE = [0, 512, 1408, 2048]
KEDGE = [0, 16, 42, 64]

BF16 = ml_dtypes.bfloat16

# Exposed for test harnesses
LAST_RESULTS = []
LAST_EXEC_NS = None

_cache = {}


def _prep(king_positions):
    """Group the 2048 (sample, slot) pairs by king into a 2048-long column
    order; split groups at psum-band (512) and DMA-piece edges."""
    kings = np.asarray(king_positions).astype(np.int64)  # (B, 2)
    groups = [[] for _ in range(K)]
    for b in range(B):
        groups[kings[b, 0]].append((b, 0))
        groups[kings[b, 1]].append((b, 1))

    edges = sorted(set([b * BINW for b in range(NBAND + 1)] + FEDGE))

    order = []       # col -> (b, s)
    subgroups = []   # (king, c0, n)
    col = 0
    for k in range(K):
        g = groups[k]
        i = 0
        while i < len(g):
            nxt = min(e for e in edges if e > col)
            n = min(len(g) - i, nxt - col)
            subgroups.append((k, col, n))
            order.extend(g[i : i + n])
            i += n
            col += n
    assert col == NCOL

    pos = np.empty((B, 2), dtype=np.int64)
    for c, (b, s) in enumerate(order):
        pos[b, s] = c
    return order, tuple(subgroups), pos


def _build(subgroups):
    nc = bacc.Bacc(
        "TRN2", target_bir_lowering=False, debug=False, num_devices=NCORES
    )
    dt = mybir.dt

    w_ins = [
        nc.dram_tensor(
            f"w_in{i}", [P, KEDGE[i + 1] - KEDGE[i], FCH, CH], dt.bfloat16,
            kind="ExternalInput",
        )
        for i in range(NPIECE)
    ]
    f_ins = [
        nc.dram_tensor(
            f"f_in{i}", [P, FCH, FEDGE[i + 1] - FEDGE[i]], dt.bfloat16,
            kind="ExternalInput",
        )
        for i in range(NPIECE)
    ]
    koh_d = nc.dram_tensor("koh", [K + 1, NCOL], dt.bfloat16, kind="ExternalInput")
    eb65_d = nc.dram_tensor("eb65", [K + 1, CH], dt.bfloat16, kind="ExternalInput")
    sof_d = nc.dram_tensor("sof", [P, NCHUNK], dt.float16, kind="ExternalInput")
    outT = nc.dram_tensor("outT", [CH, B], dt.float32, kind="ExternalOutput")

    with tile.TileContext(nc) as tc, ExitStack() as ctx:
        const_pool = ctx.enter_context(tc.tile_pool(name="const", bufs=1))
        psum_pool = ctx.enter_context(tc.tile_pool(name="psum", bufs=1, space="PSUM"))

        # latency-critical smalls + first weight piece on the sync HWDGE
        # queue (otherwise idle); the iota + later weight pieces ride the
        # SWDGE queue (slower drain, needed later)
        sof16_sb = const_pool.tile([P, NCHUNK], dt.float16)
        nc.gpsimd.dma_start(out=sof16_sb[:], in_=sof_d.ap())
        srow_sb = const_pool.tile([P, B], dt.float32)
        nc.gpsimd.iota(
            srow_sb[:], pattern=[[1, B]], base=0, channel_multiplier=0,
            allow_small_or_imprecise_dtypes=True,
        )
        w_sb = []
        for i in range(NPIECE):
            nk = KEDGE[i + 1] - KEDGE[i]
            wt = const_pool.tile([P, nk * FCH * CH], dt.bfloat16, name=f"wt{i}")
            (nc.sync if i == 0 else nc.gpsimd).dma_start(
                out=wt[:], in_=w_ins[i].ap().rearrange("p k f c -> p (k f c)")
            )
            w_sb.append(wt)
        koh_sb = const_pool.tile([K + 1, NCOL], dt.bfloat16)
        nc.sync.dma_start(out=koh_sb[:], in_=koh_d.ap())
        eb65_sb = const_pool.tile([K + 1, CH], dt.bfloat16)
        nc.sync.dma_start(out=eb65_sb[:], in_=eb65_d.ap())
        ident_sb = const_pool.tile([CH, CH], dt.bfloat16)
        make_identity(nc, ident_sb[:])
        nsof_sb = const_pool.tile([P, NCHUNK], dt.float32)
        srow16_sb = const_pool.tile([P, B], dt.float16)
        with tc.high_priority():
            nc.vector.tensor_scalar(
                out=nsof_sb[:], in0=sof16_sb[:], scalar1=-1.0, scalar2=None,
                op0=mybir.AluOpType.mult,
            )
            nc.vector.tensor_copy(srow16_sb[:], srow_sb[:])

        # one-hot pairing tiles S_c[p, b] = (sampleof(128c+p) == b), bf16
        s_sb = const_pool.tile([P, NCHUNK * B], dt.bfloat16)
        sact_sb = const_pool.tile([P, B], dt.float32)

        def emit_sgen_dve(cs):
            for c in cs:
              with tc.high_priority():
                nc.vector.tensor_tensor(
                    out=s_sb[:, c * B : (c + 1) * B],
                    in0=sof16_sb[:, c : c + 1].to_broadcast([P, B]),
                    in1=srow16_sb[:],
                    op=mybir.AluOpType.is_equal,
                )

        def emit_sgen_act(cs):
            # S_c = relu(1 - |srow - sof_c|): exact one-hot for integers
            for c in cs:
                nc.scalar.activation(
                    out=sact_sb[:],
                    in_=srow_sb[:],
                    func=mybir.ActivationFunctionType.Abs,
                    bias=nsof_sb[:, c : c + 1],
                    scale=1.0,
                )
                nc.scalar.activation(
                    out=s_sb[:, c * B : (c + 1) * B],
                    in_=sact_sb[:],
                    func=mybir.ActivationFunctionType.Relu,
                    bias=1.0,
                    scale=-1.0,
                )

        # feature stream on the two HWDGE queues (triggers must precede the
        # ACT S-gen ops in the scalar sequencer's FIFO)
        f_sb = []
        for i in range(NPIECE):
            nf = FEDGE[i + 1] - FEDGE[i]
            ft = const_pool.tile([P, FCH * nf], dt.bfloat16, name=f"ft{i}")
            nc.scalar.dma_start(
                out=ft[:], in_=f_ins[i].ap().rearrange("p f c -> p (f c)")
            )
            f_sb.append(ft)



        # psum: main accumulators (one bank per band; band b uses
        # partitions [32b, 32b+32) so its matmuls col-tile concurrently),
        # transpose staging, pairing output
        accs = [
            psum_pool.tile([P, BINW], dt.float32, space="PSUM", name=f"acc{i}")
            for i in range(NBAND)
        ]
        tpsum = psum_pool.tile([P, NCHUNK * CH], dt.bfloat16, space="PSUM")
        pout = psum_pool.tile([P, B], dt.float32, space="PSUM")

        rows_sb = const_pool.tile([CH, NCOL], dt.bfloat16)
        rcT_sb = const_pool.tile([P, NCHUNK * CH], dt.bfloat16)

        def piece_of(c0):
            for i in range(NPIECE):
                if FEDGE[i] <= c0 < FEDGE[i + 1]:
                    return i
            raise AssertionError

        by_piece_band = {}
        for k, c0, n in subgroups:
            by_piece_band.setdefault(
                (piece_of(c0), c0 // BINW), []
            ).append((k, c0, n))

        band_started = [False] * NBAND

        def emit_mains(pc):
            bands = sorted(b for (p_, b) in by_piece_band if p_ == pc)
            lists = [list(by_piece_band[(pc, b)]) for b in bands]
            li = 0
            while any(lists):
                if lists[li % len(lists)]:
                    k, c0, n = lists[li % len(lists)].pop(0)
                    band = c0 // BINW
                    off = c0 % BINW
                    poff = c0 - FEDGE[pc]
                    npc = FEDGE[pc + 1] - FEDGE[pc]
                    wc = next(
                        i for i in range(NPIECE) if KEDGE[i] <= k < KEDGE[i + 1]
                    )
                    kk = k - KEDGE[wc]
                    for ch in range(FCH):
                        st = not band_started[band]
                        band_started[band] = True
                        nc.tensor.matmul(
                            out=accs[band][32 * band : 32 * band + CH, off : off + n],
                            lhsT=w_sb[wc][
                                :, (kk * FCH + ch) * CH : (kk * FCH + ch + 1) * CH
                            ],
                            rhs=f_sb[pc][:, ch * npc + poff : ch * npc + poff + n],
                            start=st,
                            stop=False,
                            tile_position=(0, 32 * band),
                        )
                li += 1

        def emit_fold(band):
            # psum band -> flat bf16 rows (partition-shifted psum read)
            nc.vector.tensor_copy(
                rows_sb[:, band * BINW : (band + 1) * BINW],
                accs[band][32 * band : 32 * band + CH, :],
            )

        def emit_transposes(cs):
            for c in cs:
                nc.tensor.transpose(
                    out=tpsum[:, c * CH : (c + 1) * CH],
                    in_=rows_sb[:, c * P : (c + 1) * P],
                    identity=ident_sb[:],
                )

        def emit_tcopy(g):
            nc.vector.tensor_copy(
                rcT_sb[:, g * 4 * CH : (g + 1) * 4 * CH],
                tpsum[:, g * 4 * CH : (g + 1) * 4 * CH],
            )

        def emit_pairing(cs):
            for c in cs:
                band = c % NBAND
                for h in range(2):
                    nc.tensor.matmul(
                        out=pout[32 * band : 32 * band + CH, h * BINW : (h + 1) * BINW],
                        lhsT=rcT_sb[:, c * CH : (c + 1) * CH],
                        rhs=s_sb[:, c * B + h * BINW : c * B + (h + 1) * BINW],
                        start=(c // NBAND == 0),
                        stop=(c // NBAND == NBAND - 1),
                        tile_position=(0, 32 * band),
                        skip_group_check=True,
                    )

        def emit_extras(bands):
            # row-640 extra (all pairs) + bias (slot-A pairs): one K=65
            # matmul per band closes the band's accumulation
            for band in bands:
                nc.tensor.matmul(
                    out=accs[band][32 * band : 32 * band + CH, :],
                    lhsT=eb65_sb[:, :],
                    rhs=koh_sb[:, band * BINW : (band + 1) * BINW],
                    start=False,
                    stop=True,
                    tile_position=(0, 32 * band),
                )

        # pipeline: mains by piece; band work as its columns complete
        emit_sgen_act(range(11, NCHUNK))   # ACT, parallel to everything
        emit_sgen_dve(range(0, 6))
        emit_mains(0)
        emit_extras([0])
        emit_fold(0)
        emit_sgen_dve(range(6, 8))
        emit_mains(1)
        emit_extras([1])
        emit_fold(1)
        emit_transposes(range(0, 8))
        emit_tcopy(0)
        emit_sgen_dve(range(8, 11))
        emit_tcopy(1)
        emit_pairing(range(0, 8))
        emit_mains(2)
        emit_extras([2, 3])
        emit_fold(2)
        emit_fold(3)
        emit_transposes(range(8, 16))
        emit_tcopy(2)
        emit_tcopy(3)
        emit_pairing(range(8, 16))

        # sum the 4 pairing bands (partition-shifted psum reads)
        t0_sb = const_pool.tile([CH, B], dt.bfloat16)
        nc.scalar.copy(t0_sb[:], pout[0:CH, :])
        t1_sb = const_pool.tile([CH, B], dt.bfloat16)
        nc.vector.tensor_tensor(
            out=t1_sb[:], in0=pout[CH : 2 * CH, :], in1=t0_sb[:],
            op=mybir.AluOpType.add,
        )
        t2_sb = const_pool.tile([CH, B], dt.bfloat16)
        nc.vector.tensor_tensor(
            out=t2_sb[:], in0=pout[2 * CH : 3 * CH, :], in1=t1_sb[:],
            op=mybir.AluOpType.add,
        )
        outT_sb = const_pool.tile([CH, B], dt.float32)
        nc.vector.tensor_tensor(
            out=outT_sb[:], in0=pout[3 * CH : 4 * CH, :], in1=t2_sb[:],
            op=mybir.AluOpType.add,
        )
        nc.sync.dma_start(out=outT.ap(), in_=outT_sb[:])

    nc.compile()
    return nc


def kernel(piece_positions, king_positions, input_weights, bias):
    global LAST_RESULTS, LAST_EXEC_NS

    p_flat = np.asarray(piece_positions).reshape(B, F)
    w_full = np.ascontiguousarray(np.asarray(input_weights), dtype=np.float32)
    bias_np = np.asarray(bias, dtype=np.float32)

    order, subgroups, pos = _prep(king_positions)

    if subgroups not in _cache:
        _cache[subgroups] = _build(subgroups)
    nc = _cache[subgroups]

    w_bf = w_full.astype(BF16)  # (K, 641, C)

    # features in pair-column order: (2048, 640) -> (128, 5, 2048) bf16
    bs = np.array([b for b, _ in order], dtype=np.int64)
    featsT = (
        p_flat[bs].astype(np.float32).reshape(NCOL, FCH, 128).transpose(2, 1, 0)
    ).astype(BF16)
    f_pieces = [
        np.ascontiguousarray(featsT[:, :, FEDGE[i] : FEDGE[i + 1]])
        for i in range(NPIECE)
    ]

    # koh: one-hot king per column (row-640 extra) + slot-A row (bias once)
    koh = np.zeros((K + 1, NCOL), dtype=np.float32)
    for c0, (b, s) in enumerate(order):
        koh[np.asarray(king_positions)[b, s], c0] = 1.0
        if s == 0:
            koh[K, c0] = 1.0
    koh = koh.astype(BF16)

    # pairing metadata: sample index of each pair column, and iota row
    sof16 = np.empty((P, NCHUNK), dtype=np.float16)
    for c0, (b, s) in enumerate(order):
        sof16[c0 % P, c0 // P] = float(b)

    in_maps = []
    for c in range(NCORES):
        chs = slice(c * CH, (c + 1) * CH)
        w_c = (
            w_bf[:, :F, chs].reshape(K, FCH, 128, CH).transpose(2, 0, 1, 3)
        )  # (128, K, FCH, CH)
        eb65 = np.zeros((K + 1, CH), dtype=np.float32)
        eb65[:K] = w_full[:, F, chs]
        eb65[K] = bias_np[chs]
        m = {
            "koh": koh,
            "eb65": np.ascontiguousarray(eb65).astype(BF16),
            "sof": sof16,
        }
        for i in range(NPIECE):
            m[f"w_in{i}"] = np.ascontiguousarray(w_c[:, KEDGE[i] : KEDGE[i + 1]])
            m[f"f_in{i}"] = f_pieces[i]
        in_maps.append(m)

    do_trace = bool(int(os.environ.get("KERNEL_TRACE", "0")))
    trace_kw = dict(
        trace=do_trace, trace_cores=list(range(NCORES)) if do_trace else None
    )

    res = run_bass_kernel_spmd(nc, in_maps, list(range(NCORES)), **trace_kw)

    LAST_RESULTS = [res]
    LAST_EXEC_NS = res.exec_time_ns

    out = np.empty((B, C), dtype=np.float32)
    for c in range(NCORES):
        out[:, c * CH : (c + 1) * CH] = res.results[c]["outT"].T
    return out
